# revision 17
# baseline (speedup 1.0000x reference)
"""Trainium2 Bass kernel for nn_MinimalQuantumLayer.

Math: the reference simulates a fixed 4-qubit circuit (RY encoding of a
2x2 patch, then 2 layers of [RX(w_q) on each qubit + CNOT ring]) and
measures <Z_q>.  In the Heisenberg picture O_q = C^dag Z_q C expands in
the Pauli basis; for a product state RY(theta_i)|0> the per-qubit
expectations are <Z>=cos(theta), <X>=sin(theta), <Y>=0, so every Pauli
string containing a Y drops out.  Only 12 strings survive (2/2/4/4 for
q=0..3), with weight-dependent scalar coefficients that are computed on
the host from the 16x16 circuit unitary:

  out0 = C0*C1*C3 * (a00 + a01*S2)
  out1 = C0*C2*C3 * (a10 + a11*S1)
  out2 = C1*C3 * ((b0 + b1*S0) + S2*(b2 + b3*S0))
  out3 = C0*C2 * ((d0 + d1*S1) + S3*(d2 + d3*S1))

with C_i = cos(pi/2 * x_i), S_i = sin(pi/2 * x_i) over the 4 pixels of
each 2x2 patch (qubit 0=(r0,c0), 1=(r0,c1), 2=(r1,c0), 3=(r1,c1)).

The device kernel is purely elementwise in patch-major layout
(partition = patch row, free dim = (image, patch col)): 8 ScalarE Sin
activations (cos via sin(u + pi/2)) + 18 VectorE ops per core.
Data-parallel over the batch: each of the 8 cores takes 4 images.
Coefficients are passed as a runtime input tensor so the NEFF does not
depend on the weight values.
"""

import numpy as np

from concourse import bacc, bass, mybir
from concourse.bass_utils import run_bass_kernel_spmd
from concourse.tile import TileContext

N_CORES = 8
B_TOTAL = 32
B_PER = B_TOTAL // N_CORES  # 4 images per core
H = W = 256
OH = OW = 128
FD = B_PER * OW  # 512: free dim of the patch-major compute tiles
F32 = mybir.dt.float32
PI_2 = float(np.pi / 2)


# ---------------------------------------------------------------- host math
def _pauli_coefs(w: np.ndarray) -> np.ndarray:
    """The 12 surviving Pauli coefficients of C^dag Z_q C, from q_weights."""
    I2 = np.eye(2, dtype=complex)
    X = np.array([[0, 1], [1, 0]], dtype=complex)
    Z = np.array([[1, 0], [0, -1]], dtype=complex)

    def kron_list(ms):
        out = np.array([[1.0 + 0j]])
        for m in ms:
            out = np.kron(out, m)
        return out

    def op_on(U, q):
        ms = [I2] * 4
        ms[q] = U
        return kron_list(ms)

    def cnot(c, t):
        M = np.zeros((16, 16), dtype=complex)
        for k in range(16):
            bits = [(k >> (3 - i)) & 1 for i in range(4)]
            if bits[c] == 1:
                bits[t] ^= 1
            k2 = 0
            for b in bits:
                k2 = (k2 << 1) | b
            M[k2, k] = 1
        return M

    C = np.eye(16, dtype=complex)
    for l in range(w.shape[0]):
        for q in range(4):
            c, s = np.cos(w[l, q] * 0.5), np.sin(w[l, q] * 0.5)
            C = op_on(np.array([[c, -1j * s], [-1j * s, c]]), q) @ C
        for q in range(4):
            C = cnot(q, (q + 1) % 4) @ C

    mats = {"I": I2, "X": X, "Z": Z}
    support = [
        (0, "ZZIZ"), (0, "ZZXZ"),
        (1, "ZIZZ"), (1, "ZXZZ"),
        (2, "IZIZ"), (2, "XZIZ"), (2, "IZXZ"), (2, "XZXZ"),
        (3, "ZIZI"), (3, "ZXZI"), (3, "ZIZX"), (3, "ZXZX"),
    ]
    obs = {q: C.conj().T @ op_on(Z, q) @ C for q in range(4)}
    coefs = np.empty(len(support), dtype=np.float64)
    for i, (q, s) in enumerate(support):
        P = kron_list([mats[ch] for ch in s])
        coefs[i] = (np.trace(P.conj().T @ obs[q]) / 16).real
    return coefs


# ---------------------------------------------------------------- device IR
def _build_nc() -> bass.Bass:
    nc = bacc.Bacc(
        "TRN2", target_bir_lowering=False, debug=False, num_devices=N_CORES
    )
    x = nc.dram_tensor("x", [B_PER, H, W], F32, kind="ExternalInput")
    coef = nc.dram_tensor("coef", [128, 16], F32, kind="ExternalInput")
    out = nc.dram_tensor("out", [B_PER, OH, 4 * OW], F32, kind="ExternalOutput")

    Sin = mybir.ActivationFunctionType.Sin
    Ident = mybir.ActivationFunctionType.Identity
    mul = mybir.AluOpType.mult
    add = mybir.AluOpType.add

    with TileContext(nc) as tc:
        with tc.tile_pool(name="p", bufs=1) as pool:
            coef_t = pool.tile([128, 16], F32, tag="coef")
            nc.sync.dma_start(out=coef_t[:, :], in_=coef[:, :])

            # One fully-contiguous 1MB DMA: partition = patch row (row
            # pair), free = (image, row parity, col) -- 2KB contiguous
            # chunks, 100% HBM efficiency.
            t_all = pool.tile([128, B_PER * 2 * W], F32, tag="t_all")
            nc.sync.dma_start(
                out=t_all[:, :].rearrange("p (b t w) -> p b t w", b=B_PER, t=2),
                in_=x[:, :, :].rearrange("b (p t) w -> p b t w", t=2),
            )
            t_view = t_all[:, :].rearrange(
                "p (b t w) -> p b t w", b=B_PER, t=2
            )

            def trig(parity, col_off, is_cos, tag):
                # cos(u) = sin(u + pi/2); pi/2 bias comes from coef[:,12]
                t = pool.tile([128, FD], F32, tag=tag, name=tag)
                nc.scalar.activation(
                    t[:, :],
                    t_view[:, :, parity, col_off::2],
                    Sin,
                    bias=coef_t[:, 12:13] if is_cos else 0.0,
                    scale=PI_2,
                )
                return t

            # qubit 0=(even,even) 1=(even,odd) 2=(odd,even) 3=(odd,odd)
            # Order: unblock P13/M0/P02/M1 (VectorE) as early as possible.
            c3 = trig(1, 1, True, "c3")
            c1 = trig(0, 1, True, "c1")
            c0 = trig(0, 0, True, "c0")
            c2 = trig(1, 0, True, "c2")
            s2 = trig(1, 0, False, "s2")
            s1 = trig(0, 1, False, "s1")
            s0 = trig(0, 0, False, "s0")
            s3 = trig(1, 1, False, "s3")

            def tile(tag):
                return pool.tile([128, FD], F32, tag=tag, name=tag)

            def cf(i):  # per-partition scalar AP for runtime coefficient i
                return coef_t[:, i : i + 1]

            out_t = pool.tile([128, 4 * FD], F32, tag="out")

            p13 = tile("p13")
            nc.vector.tensor_tensor(out=p13[:, :], in0=c1[:, :], in1=c3[:, :], op=mul)
            p02 = tile("p02")
            nc.vector.tensor_tensor(out=p02[:, :], in0=c0[:, :], in1=c2[:, :], op=mul)
            m0 = tile("m0")
            nc.vector.tensor_tensor(out=m0[:, :], in0=c0[:, :], in1=p13[:, :], op=mul)
            m1 = tile("m1")
            nc.vector.tensor_tensor(out=m1[:, :], in0=c3[:, :], in1=p02[:, :], op=mul)

            # a0 = a00 + a01*S2 ; a1 = a10 + a11*S1
            a0 = tile("a0")
            nc.scalar.activation(
                a0[:, :], s2[:, :], Ident, bias=cf(0), scale=cf(1)
            )
            a1 = tile("a1")
            nc.scalar.activation(
                a1[:, :], s1[:, :], Ident, bias=cf(2), scale=cf(3)
            )
            nc.vector.tensor_tensor(
                out=out_t[:, 0 :: 4], in0=m0[:, :], in1=a0[:, :], op=mul
            )
            nc.vector.tensor_tensor(
                out=out_t[:, 1 :: 4], in0=m1[:, :], in1=a1[:, :], op=mul
            )

            # out2 = P13*((b0+b1*S0) + S2*(b2+b3*S0))
            b1t = tile("b1t")
            nc.scalar.activation(
                b1t[:, :], s0[:, :], Ident, bias=cf(4), scale=cf(5)
            )
            b2t = tile("b2t")
            nc.scalar.activation(
                b2t[:, :], s0[:, :], Ident, bias=cf(6), scale=cf(7)
            )
            b3t = tile("b3t")
            nc.vector.tensor_tensor(out=b3t[:, :], in0=s2[:, :], in1=b2t[:, :], op=mul)
            b4t = tile("b4t")
            nc.vector.tensor_tensor(out=b4t[:, :], in0=b1t[:, :], in1=b3t[:, :], op=add)
            nc.vector.tensor_tensor(
                out=out_t[:, 2 :: 4], in0=p13[:, :], in1=b4t[:, :], op=mul
            )

            # out3 = P02*((d0+d1*S1) + S3*(d2+d3*S1))
            d1t = tile("d1t")
            nc.scalar.activation(
                d1t[:, :], s1[:, :], Ident, bias=cf(8), scale=cf(9)
            )
            d2t = tile("d2t")
            nc.scalar.activation(
                d2t[:, :], s1[:, :], Ident, bias=cf(10), scale=cf(11)
            )
            d3t = tile("d3t")
            nc.vector.tensor_tensor(out=d3t[:, :], in0=s3[:, :], in1=d2t[:, :], op=mul)
            d4t = tile("d4t")
            nc.vector.tensor_tensor(out=d4t[:, :], in0=d1t[:, :], in1=d3t[:, :], op=add)
            nc.vector.tensor_tensor(
                out=out_t[:, 3 :: 4], in0=p02[:, :], in1=d4t[:, :], op=mul
            )

            nc.sync.dma_start(
                out=out[:, :, :].rearrange("b p w -> p b w"),
                in_=out_t[:, :].rearrange("p (b w) -> p b w", b=B_PER),
            )
    nc.compile()
    return nc


_NC_CACHE = None


def _get_nc() -> bass.Bass:
    global _NC_CACHE
    if _NC_CACHE is None:
        _NC_CACHE = _build_nc()
    return _NC_CACHE


# ---------------------------------------------------------------- entry point
def kernel(x: np.ndarray, q_weights: np.ndarray, _trace: bool = False):
    coefs = _pauli_coefs(np.asarray(q_weights, dtype=np.float64))
    coef_tile = np.zeros((128, 16), dtype=np.float32)
    coef_tile[:, : len(coefs)] = coefs.astype(np.float32)
    coef_tile[:, 12] = np.float32(PI_2)

    xs = np.ascontiguousarray(
        np.asarray(x, dtype=np.float32).reshape(B_TOTAL, H, W)
    )
    in_maps = [
        {"x": xs[B_PER * c : B_PER * (c + 1)], "coef": coef_tile}
        for c in range(N_CORES)
    ]
    nc = _get_nc()
    res = run_bass_kernel_spmd(
        nc, in_maps, core_ids=list(range(N_CORES)), trace=_trace
    )
    out = np.concatenate(
        [res.results[c]["out"].reshape(B_PER, OH, OW, 4) for c in range(N_CORES)],
        axis=0,
    )
    if _trace:
        return out, res
    return out
